# revision 1
# baseline (speedup 1.0000x reference)
"""TRN2 Bass kernel for nn_CircularNN (gnn_message_passing).

Network (reference):
    h = gelu(sp1(x));  h = bn2(gelu(sp2(h)));  h = bn3(gelu(sp3(h)))
    h = bn2'(gelu(sp2(h)));  out = relu(sp1(h))
where sp(x)[b,j] = sum_k x[b, idx[j,k]] * W[j,k] + b[j] and bn* are
BatchNorm1d training-mode (batch statistics over the full 16384 batch).

Strategy:
  - Pure data parallel over 8 cores: each core handles 2048 rows of x.
  - The sparse gather op is expressed as a dense matmul with a host-built
    scatter matrix S[i, j] = sum_k W[j,k] * [idx[j,k] == i]  (784x784,
    padded to 896x896 = 7 partition tiles of 128).
  - Activations live feature-major on chip: [features(part) x batch(free)],
    so BN stats are free-dim reductions (bn_stats/bn_aggr) and the
    per-feature affine is a per-partition tensor_scalar.  Host transposes
    x/out (outside the measured NEFF).
  - Matmuls run in float32r (full PE speed at free-dim 512, ~2e-4 rel err).
  - Sync-BN: per-core [mean, E[x^2]] partial stats (2x896 floats) are
    AllReduce-summed across the 8 cores, then alpha/delta are computed
    per-feature and applied on-chip.
"""

import numpy as np

import concourse.bass as bass  # noqa: F401
import concourse.tile as tile
from concourse import bacc, mybir
from concourse.bass_utils import run_bass_kernel_spmd

NCORES = 8
B, NREAL = 16384, 784
P = 128
NT = 7                   # feature tiles (896 = 7*128)
NF = NT * P              # padded feature count
BLOC = B // NCORES       # rows per core
CH = 512                 # batch chunk (free dim per matmul / bn_stats)
NCH = BLOC // CH
BN_EPS = 1e-5

F32 = mybir.dt.float32
F32R = mybir.dt.float32r
AF = mybir.ActivationFunctionType
ALU = mybir.AluOpType

VEC_NAMES = ["b1", "b2", "b3", "g2", "be2", "g3", "be3"]

# (scatter-matrix index, bias name, activation, (gamma, beta) or None)
LAYERS = [
    (0, "b1", AF.Gelu, None),
    (1, "b2", AF.Gelu, ("g2", "be2")),
    (2, "b3", AF.Gelu, ("g3", "be3")),
    (1, "b2", AF.Gelu, ("g2", "be2")),
    (0, "b1", AF.Relu, None),
]

_NC_CACHE = {}


def _build_nc():
    nc = bacc.Bacc("TRN2", target_bir_lowering=False, debug=False, num_devices=NCORES)

    xT_d = nc.dram_tensor("xT", [NF, BLOC], F32R, kind="ExternalInput").ap()
    s_d = [
        nc.dram_tensor(f"s{i}", [NF, NF], F32R, kind="ExternalInput").ap()
        for i in range(3)
    ]
    vec_d = {
        nm: nc.dram_tensor(nm, [P, NT], F32, kind="ExternalInput").ap()
        for nm in VEC_NAMES
    }
    outT_d = nc.dram_tensor("outT", [NF, BLOC], F32, kind="ExternalOutput").ap()

    with tile.TileContext(nc) as tc:
        with (
            tc.tile_pool(name="acts", bufs=1) as acts,
            tc.tile_pool(name="spool", bufs=1) as spool,
            tc.tile_pool(name="consts", bufs=1) as consts,
            tc.tile_pool(name="small", bufs=2) as small,
            tc.tile_pool(name="psum", bufs=6, space="PSUM") as psum,
            tc.tile_pool(name="dram", bufs=2, space="DRAM") as dram,
        ):
            # per-feature vectors, one [128, NT] tile each
            vec_t = {}
            for nm in VEC_NAMES:
                t = consts.tile([P, NT], F32, tag=f"v_{nm}")
                nc.sync.dma_start(out=t, in_=vec_d[nm])
                vec_t[nm] = t

            # input activations (xT), chunked DMA so matmuls can start early
            h_in = acts.tile([P, NT, BLOC], F32R, tag="hA")
            for c in range(NCH):
                for k in range(NT):
                    nc.sync.dma_start(
                        out=h_in[:, k, c * CH:(c + 1) * CH],
                        in_=xT_d[k * P:(k + 1) * P, c * CH:(c + 1) * CH],
                    )

            # scatter-matrix tiles: explicit ping-pong (tags sA/sB).
            # Load order: s0->A, s1->B, s2->A, (reuse s1 in B), s0->A again.
            s_tiles = {}

            def load_s(idx, tag):
                t = spool.tile([P, NT, NF], F32R, tag=tag)
                for k in range(NT):
                    nc.sync.dma_start(
                        out=t[:, k, :], in_=s_d[idx][k * P:(k + 1) * P, :]
                    )
                return t

            act_tag = {0: "hA", 1: "hB"}
            s_tag_cycle = ["sA", "sB", "sA", None, "sA"]  # None -> reuse cached

            for li, (sidx, bias_nm, act_fn, bn) in enumerate(LAYERS):
                last = li == len(LAYERS) - 1
                if s_tag_cycle[li] is not None:
                    s_tiles[li] = load_s(sidx, s_tag_cycle[li])
                    if li == 1:
                        s_reuse = s_tiles[li]  # s1 stays resident in sB for layer 3
                else:
                    s_tiles[li] = s_reuse
                s_t = s_tiles[li]

                out_dt = F32 if last else F32R
                h_out = acts.tile([P, NT, BLOC], out_dt, tag=act_tag[(li + 1) % 2])

                if bn is not None:
                    stats = small.tile([P, NT, NCH, 6], F32, tag="stats")

                bias_t = vec_t[bias_nm]
                for c in range(NCH):
                    csl = slice(c * CH, (c + 1) * CH)
                    for j in range(NT):
                        pt = psum.tile([P, CH], F32, tag="ps")
                        for k in range(NT):
                            nc.tensor.matmul(
                                pt,
                                lhsT=s_t[:, k, j * P:(j + 1) * P],
                                rhs=h_in[:, k, csl],
                                start=(k == 0),
                                stop=(k == NT - 1),
                            )
                        nc.scalar.activation(
                            out=h_out[:, j, csl],
                            in_=pt,
                            func=act_fn,
                            bias=bias_t[:, j:j + 1],
                            scale=1.0,
                        )
                        if bn is not None:
                            nc.vector.bn_stats(
                                out=stats[:, j, c, :],
                                in_=h_out[:, j, csl].bitcast(F32),
                            )
                        if last:
                            nc.sync.dma_start(
                                out=outT_d[j * P:(j + 1) * P, csl],
                                in_=h_out[:, j, csl],
                            )

                if bn is not None:
                    g_nm, be_nm = bn
                    mvall = small.tile([P, NT, 2], F32, tag="mvall")
                    for j in range(NT):
                        nc.vector.bn_aggr(out=mvall[:, j, :], in_=stats[:, j, :, :])
                    # per-core partial sums: [mean_j, E[x^2]_j] laid out [P, 2, NT]
                    sums = small.tile([P, 2, NT], F32, tag="sums")
                    nc.vector.tensor_copy(out=sums[:, 0, :], in_=mvall[:, :, 0])
                    nc.vector.tensor_tensor(
                        out=sums[:, 1, :], in0=mvall[:, :, 0], in1=mvall[:, :, 0],
                        op=ALU.mult,
                    )
                    nc.vector.tensor_tensor(
                        out=sums[:, 1, :], in0=sums[:, 1, :], in1=mvall[:, :, 1],
                        op=ALU.add,
                    )
                    inb = dram.tile([P, 2 * NT], F32, tag="inb")
                    outb = dram.tile([P, 2 * NT], F32, tag="outb", addr_space="Shared")
                    nc.sync.dma_start(out=inb, in_=sums[:, :, :])
                    nc.gpsimd.collective_compute(
                        "AllReduce",
                        ALU.add,
                        replica_groups=[list(range(NCORES))],
                        ins=[inb.opt()],
                        outs=[outb.opt()],
                    )
                    gsums = small.tile([P, 2, NT], F32, tag="gsums")
                    nc.sync.dma_start(out=gsums[:, :, :], in_=outb)

                    gm = small.tile([P, NT], F32, tag="gm")
                    gv = small.tile([P, NT], F32, tag="gv")
                    tmp = small.tile([P, NT], F32, tag="tmp")
                    alpha = small.tile([P, NT], F32, tag="alpha")
                    delta = small.tile([P, NT], F32, tag="delta")
                    inv_n = 1.0 / NCORES
                    nc.vector.tensor_scalar(
                        out=gm, in0=gsums[:, 0, :], scalar1=inv_n, scalar2=None,
                        op0=ALU.mult,
                    )
                    nc.vector.tensor_scalar(
                        out=gv, in0=gsums[:, 1, :], scalar1=inv_n, scalar2=None,
                        op0=ALU.mult,
                    )
                    nc.vector.tensor_tensor(out=tmp, in0=gm, in1=gm, op=ALU.mult)
                    nc.vector.tensor_tensor(out=gv, in0=gv, in1=tmp, op=ALU.subtract)
                    nc.vector.tensor_scalar(
                        out=gv, in0=gv, scalar1=BN_EPS, scalar2=None, op0=ALU.add,
                    )
                    nc.vector.reciprocal(out=gv, in_=gv)
                    nc.scalar.activation(out=gv, in_=gv, func=AF.Sqrt)  # rstd
                    nc.vector.tensor_tensor(
                        out=alpha, in0=gv, in1=vec_t[g_nm], op=ALU.mult
                    )
                    nc.vector.tensor_tensor(out=tmp, in0=gm, in1=alpha, op=ALU.mult)
                    nc.vector.tensor_tensor(
                        out=delta, in0=vec_t[be_nm], in1=tmp, op=ALU.subtract
                    )

                    # apply BN in place: h = alpha * h + delta  (Pool engine)
                    for c in range(NCH):
                        csl = slice(c * CH, (c + 1) * CH)
                        for j in range(NT):
                            nc.gpsimd.tensor_scalar(
                                out=h_out[:, j, csl],
                                in0=h_out[:, j, csl],
                                scalar1=alpha[:, j:j + 1],
                                scalar2=delta[:, j:j + 1],
                                op0=ALU.mult,
                                op1=ALU.add,
                            )

                h_in = h_out

    nc.compile()
    return nc


def _get_nc():
    if "nc" not in _NC_CACHE:
        _NC_CACHE["nc"] = _build_nc()
    return _NC_CACHE["nc"]


def _scatter_matrix(idx, W):
    """S[i, j] = sum_k W[j, k] * [idx[j, k] == i], padded to [NF, NF]."""
    S = np.zeros((NF, NF), np.float32)
    n, K = idx.shape
    cols = np.arange(n)
    for k in range(K):
        np.add.at(S, (idx[:, k].astype(np.int64), cols), W[:, k])
    return S


def _vec_tile(v):
    p = np.zeros(NF, np.float32)
    p[:NREAL] = v
    return np.ascontiguousarray(p.reshape(NT, P).T)


def prepare_in_maps(x, idx1, W1, b1, idx2, W2, b2, idx3, W3, b3, g2, be2, g3, be3):
    x = np.asarray(x, np.float32)
    S = [
        _scatter_matrix(np.asarray(idx1), np.asarray(W1, np.float32)),
        _scatter_matrix(np.asarray(idx2), np.asarray(W2, np.float32)),
        _scatter_matrix(np.asarray(idx3), np.asarray(W3, np.float32)),
    ]
    vecs = {
        nm: _vec_tile(np.asarray(v, np.float32))
        for nm, v in zip(VEC_NAMES, [b1, b2, b3, g2, be2, g3, be3])
    }
    in_maps = []
    for c in range(NCORES):
        xT = np.zeros((NF, BLOC), np.float32)
        xT[:NREAL] = x[c * BLOC:(c + 1) * BLOC].T
        m = {"xT": np.ascontiguousarray(xT)}
        for i in range(3):
            m[f"s{i}"] = S[i]
        m.update(vecs)
        in_maps.append(m)
    return in_maps


def assemble_output(results):
    out = np.empty((B, NREAL), np.float32)
    for c in range(NCORES):
        out[c * BLOC:(c + 1) * BLOC] = results[c]["outT"][:NREAL].T
    return out


def run(in_maps, **kw):
    nc = _get_nc()
    return run_bass_kernel_spmd(nc, in_maps, core_ids=list(range(NCORES)), **kw)


def kernel(**inputs) -> np.ndarray:
    in_maps = prepare_in_maps(**inputs)
    res = run(in_maps)
    return assemble_output(res.results)


# revision 6
# speedup vs baseline: 1.0108x; 1.0108x over previous
"""TRN2 Bass kernel for nn_CircularNN (gnn_message_passing).

Network (reference):
    h = gelu(sp1(x));  h = bn2(gelu(sp2(h)));  h = bn3(gelu(sp3(h)))
    h = bn2'(gelu(sp2(h)));  out = relu(sp1(h))
where sp(x)[b,j] = sum_k x[b, idx[j,k]] * W[j,k] + b[j] and bn* are
BatchNorm1d training-mode (batch statistics over the full 16384 batch).

Strategy:
  - Pure data parallel over 8 cores: each core handles 2048 rows of x.
  - The sparse gather op is expressed as a dense matmul with a host-built
    scatter matrix S[i, j] = sum_k W[j,k] * [idx[j,k] == i]  (784x784,
    padded to 896x896 = 7 partition tiles of 128).
  - Activations live feature-major on chip: [features(part) x batch(free)],
    so BN stats are free-dim reductions (bn_stats/bn_aggr) and the
    per-feature affine is a per-partition tensor_scalar.  Host transposes
    x/out (outside the measured NEFF).
  - Matmuls run in float32r (full PE speed at free-dim 512, ~2e-4 rel err).
  - Sync-BN: per-core [mean, E[x^2]] partial stats (2x896 floats) are
    AllReduce-summed across the 8 cores, then alpha/delta are computed
    per-feature and applied on-chip.
"""

import numpy as np

import concourse.bass as bass  # noqa: F401
import concourse.tile as tile
from concourse import bacc, mybir
from concourse.bass_utils import run_bass_kernel_spmd

NCORES = 8
B, NREAL = 16384, 784
P = 128
NT = 7                   # feature tiles (896 = 7*128)
NF = NT * P              # padded feature count
BLOC = B // NCORES       # rows per core
CH = 512                 # batch chunk (free dim per matmul / bn_stats)
NCH = BLOC // CH
BN_EPS = 1e-5

F32 = mybir.dt.float32
F32R = mybir.dt.float32r
AF = mybir.ActivationFunctionType
ALU = mybir.AluOpType

VEC_NAMES = ["b1", "b2", "b3", "g2", "be2", "g3", "be3"]

# (scatter-matrix index, bias name, activation, (gamma, beta) or None)
LAYERS = [
    (0, "b1", AF.Gelu, None),
    (1, "b2", AF.Gelu, ("g2", "be2")),
    (2, "b3", AF.Gelu, ("g3", "be3")),
    (1, "b2", AF.Gelu, ("g2", "be2")),
    (0, "b1", AF.Relu, None),
]

_NC_CACHE = {}


def _build_nc():
    nc = bacc.Bacc("TRN2", target_bir_lowering=False, debug=False, num_devices=NCORES)

    xT_d = nc.dram_tensor("xT", [NF, BLOC], F32R, kind="ExternalInput").ap()
    s_d = [
        nc.dram_tensor(f"s{i}", [NF, NF], F32R, kind="ExternalInput").ap()
        for i in range(3)
    ]
    vec_d = {
        nm: nc.dram_tensor(nm, [P, NT], F32, kind="ExternalInput").ap()
        for nm in VEC_NAMES
    }
    outT_d = nc.dram_tensor("outT", [NF, BLOC], F32, kind="ExternalOutput").ap()

    with tile.TileContext(nc) as tc:
        with (
            tc.tile_pool(name="acts", bufs=1) as acts,
            tc.tile_pool(name="spool", bufs=1) as spool,
            tc.tile_pool(name="consts", bufs=1) as consts,
            tc.tile_pool(name="small", bufs=2) as small,
            tc.tile_pool(name="psum", bufs=6, space="PSUM") as psum,
            tc.tile_pool(name="dram", bufs=2, space="DRAM") as dram,
        ):
            # scatter-matrix tiles: explicit ping-pong (tags sA/sB).
            # Load order: s0->A, s1->B, s2->A, (reuse s1 in B), s0->A again.
            # One 3D-AP DMA per matrix keeps the issue queue short.
            s_tiles = {}

            def load_s(idx, tag):
                t = spool.tile([P, NT, NF], F32R, tag=tag)
                nc.sync.dma_start(
                    out=t, in_=s_d[idx].rearrange("(t p) j -> p t j", p=P)
                )
                return t

            s_tiles[0] = load_s(0, "sA")  # layer 0 weights first: gate the ramp

            # input activations (xT): batch chunk 0 first (gates the first
            # matmuls), remaining chunks as one wide DMA per k on other queues
            h_in = acts.tile([P, NT, BLOC], F32R, tag="hA")
            for k in range(NT):
                nc.sync.dma_start(
                    out=h_in[:, k, 0:CH],
                    in_=xT_d[k * P:(k + 1) * P, 0:CH],
                )
            for k in range(NT):
                eng = (nc.scalar, nc.gpsimd)[k % 2]
                eng.dma_start(
                    out=h_in[:, k, CH:BLOC],
                    in_=xT_d[k * P:(k + 1) * P, CH:BLOC],
                )

            # per-feature vectors, one [128, NT] tile each
            vec_t = {}
            for nm in VEC_NAMES:
                t = consts.tile([P, NT], F32, tag=f"v_{nm}")
                nc.gpsimd.dma_start(out=t, in_=vec_d[nm])
                vec_t[nm] = t

            act_tag = {0: "hA", 1: "hB"}
            s_tag_cycle = ["sA", "sB", "sA", None, "sA"]  # None -> reuse cached

            for li, (sidx, bias_nm, act_fn, bn) in enumerate(LAYERS):
                last = li == len(LAYERS) - 1
                if li == 0:
                    pass  # already loaded above
                elif s_tag_cycle[li] is not None:
                    s_tiles[li] = load_s(sidx, s_tag_cycle[li])
                    if li == 1:
                        s_reuse = s_tiles[li]  # s1 stays resident in sB for layer 3
                else:
                    s_tiles[li] = s_reuse
                s_t = s_tiles[li]

                out_dt = F32 if last else F32R
                h_out = acts.tile([P, NT, BLOC], out_dt, tag=act_tag[(li + 1) % 2])

                if bn is not None:
                    stats = small.tile([P, NT, NCH, 6], F32, tag="stats")

                bias_t = vec_t[bias_nm]
                for c in range(NCH):
                    csl = slice(c * CH, (c + 1) * CH)
                    for j in range(NT):
                        pt = psum.tile([P, CH], F32, tag="ps")
                        for k in range(NT):
                            nc.tensor.matmul(
                                pt,
                                lhsT=s_t[:, k, j * P:(j + 1) * P],
                                rhs=h_in[:, k, csl],
                                start=(k == 0),
                                stop=(k == NT - 1),
                            )
                        nc.scalar.activation(
                            out=h_out[:, j, csl],
                            in_=pt,
                            func=act_fn,
                            bias=bias_t[:, j:j + 1],
                            scale=1.0,
                        )
                        if bn is not None:
                            nc.vector.bn_stats(
                                out=stats[:, j, c, :],
                                in_=h_out[:, j, csl].bitcast(F32),
                            )
                        if last:
                            nc.sync.dma_start(
                                out=outT_d[j * P:(j + 1) * P, csl],
                                in_=h_out[:, j, csl],
                            )

                if bn is not None:
                    g_nm, be_nm = bn
                    mvall = small.tile([P, NT, 2], F32, tag="mvall")
                    for j in range(NT):
                        nc.vector.bn_aggr(out=mvall[:, j, :], in_=stats[:, j, :, :])
                    # per-core partial sums: [mean_j, E[x^2]_j] laid out [P, 2, NT]
                    sums = small.tile([P, 2, NT], F32, tag="sums")
                    nc.vector.tensor_copy(out=sums[:, 0, :], in_=mvall[:, :, 0])
                    nc.vector.tensor_tensor(
                        out=sums[:, 1, :], in0=mvall[:, :, 0], in1=mvall[:, :, 0],
                        op=ALU.mult,
                    )
                    nc.vector.tensor_tensor(
                        out=sums[:, 1, :], in0=sums[:, 1, :], in1=mvall[:, :, 1],
                        op=ALU.add,
                    )
                    inb = dram.tile([P, 2 * NT], F32, tag="inb")
                    outb = dram.tile(
                        [NCORES * P, 2 * NT], F32, tag="outb", addr_space="Shared"
                    )
                    nc.sync.dma_start(out=inb, in_=sums[:, :, :])
                    nc.gpsimd.collective_compute(
                        "AllGather",
                        ALU.bypass,
                        replica_groups=[list(range(NCORES))],
                        ins=[inb.opt()],
                        outs=[outb.opt()],
                    )
                    # gather all ranks' partials and log-tree sum on DVE
                    gall = small.tile([P, NCORES, 2 * NT], F32, tag="gall")
                    nc.sync.dma_start(
                        out=gall, in_=outb.rearrange("(r p) f -> p r f", p=P)
                    )
                    g4 = small.tile([P, 4, 2 * NT], F32, tag="g4")
                    nc.vector.tensor_tensor(
                        out=g4, in0=gall[:, 0:4, :], in1=gall[:, 4:8, :], op=ALU.add
                    )
                    g2 = small.tile([P, 2, 2 * NT], F32, tag="g2t")
                    nc.vector.tensor_tensor(
                        out=g2, in0=g4[:, 0:2, :], in1=g4[:, 2:4, :], op=ALU.add
                    )
                    gsums = small.tile([P, 2, NT], F32, tag="gsums")
                    nc.vector.tensor_tensor(
                        out=gsums[:, :, :], in0=g2[:, 0, :].rearrange("p (a b) -> p a b", a=2),
                        in1=g2[:, 1, :].rearrange("p (a b) -> p a b", a=2), op=ALU.add
                    )

                    gm = small.tile([P, NT], F32, tag="gm")
                    gv = small.tile([P, NT], F32, tag="gv")
                    tmp = small.tile([P, NT], F32, tag="tmp")
                    alpha = small.tile([P, NT], F32, tag="alpha")
                    delta = small.tile([P, NT], F32, tag="delta")
                    inv_n = 1.0 / NCORES
                    nc.vector.tensor_scalar(
                        out=gm, in0=gsums[:, 0, :], scalar1=inv_n, scalar2=None,
                        op0=ALU.mult,
                    )
                    nc.vector.tensor_scalar(
                        out=gv, in0=gsums[:, 1, :], scalar1=inv_n, scalar2=None,
                        op0=ALU.mult,
                    )
                    nc.vector.tensor_tensor(out=tmp, in0=gm, in1=gm, op=ALU.mult)
                    nc.vector.tensor_tensor(out=gv, in0=gv, in1=tmp, op=ALU.subtract)
                    nc.vector.tensor_scalar(
                        out=gv, in0=gv, scalar1=BN_EPS, scalar2=None, op0=ALU.add,
                    )
                    nc.vector.reciprocal(out=gv, in_=gv)
                    nc.scalar.activation(out=gv, in_=gv, func=AF.Sqrt)  # rstd
                    nc.vector.tensor_tensor(
                        out=alpha, in0=gv, in1=vec_t[g_nm], op=ALU.mult
                    )
                    nc.vector.tensor_tensor(out=tmp, in0=gm, in1=alpha, op=ALU.mult)
                    nc.vector.tensor_tensor(
                        out=delta, in0=vec_t[be_nm], in1=tmp, op=ALU.subtract
                    )

                    # apply BN in place: h = alpha * h + delta.  Chunk 0 gates
                    # the next layer's first matmuls -> split it across DVE and
                    # Pool; trailing chunks go to Pool (off critical path).
                    for c in range(NCH):
                        csl = slice(c * CH, (c + 1) * CH)
                        for j in range(NT):
                            eng = nc.vector if (c == 0 and j % 2 == 0) else nc.gpsimd
                            eng.tensor_scalar(
                                out=h_out[:, j, csl],
                                in0=h_out[:, j, csl],
                                scalar1=alpha[:, j:j + 1],
                                scalar2=delta[:, j:j + 1],
                                op0=ALU.mult,
                                op1=ALU.add,
                            )

                h_in = h_out

    nc.compile()
    return nc


def _get_nc():
    if "nc" not in _NC_CACHE:
        _NC_CACHE["nc"] = _build_nc()
    return _NC_CACHE["nc"]


def _scatter_matrix(idx, W):
    """S[i, j] = sum_k W[j, k] * [idx[j, k] == i], padded to [NF, NF]."""
    S = np.zeros((NF, NF), np.float32)
    n, K = idx.shape
    cols = np.arange(n)
    for k in range(K):
        np.add.at(S, (idx[:, k].astype(np.int64), cols), W[:, k])
    return S


def _vec_tile(v):
    p = np.zeros(NF, np.float32)
    p[:NREAL] = v
    return np.ascontiguousarray(p.reshape(NT, P).T)


def prepare_in_maps(x, idx1, W1, b1, idx2, W2, b2, idx3, W3, b3, g2, be2, g3, be3):
    x = np.asarray(x, np.float32)
    S = [
        _scatter_matrix(np.asarray(idx1), np.asarray(W1, np.float32)),
        _scatter_matrix(np.asarray(idx2), np.asarray(W2, np.float32)),
        _scatter_matrix(np.asarray(idx3), np.asarray(W3, np.float32)),
    ]
    vecs = {
        nm: _vec_tile(np.asarray(v, np.float32))
        for nm, v in zip(VEC_NAMES, [b1, b2, b3, g2, be2, g3, be3])
    }
    in_maps = []
    for c in range(NCORES):
        xT = np.zeros((NF, BLOC), np.float32)
        xT[:NREAL] = x[c * BLOC:(c + 1) * BLOC].T
        m = {"xT": np.ascontiguousarray(xT)}
        for i in range(3):
            m[f"s{i}"] = S[i]
        m.update(vecs)
        in_maps.append(m)
    return in_maps


def assemble_output(results):
    out = np.empty((B, NREAL), np.float32)
    for c in range(NCORES):
        out[c * BLOC:(c + 1) * BLOC] = results[c]["outT"][:NREAL].T
    return out


def run(in_maps, **kw):
    nc = _get_nc()
    return run_bass_kernel_spmd(nc, in_maps, core_ids=list(range(NCORES)), **kw)


def kernel(**inputs) -> np.ndarray:
    in_maps = prepare_in_maps(**inputs)
    res = run(in_maps)
    return assemble_output(res.results)
